# revision 6
# baseline (speedup 1.0000x reference)
"""Binary KL divergence sum on 8 Trainium2 NeuronCores.

Reference math (per element, summed over all 2**25 elements):
    kl = p*(ln p - ln q) + (1-p)*(ln(1-p) - ln(1-q))

Decomposition used here (ACT does 4 logs/elem-pair; DVE ops minimized):
    L  = [ln p | ln q]          (one ACT pass over [p|q], fp32 in)
    L1 = [ln(1-p) | ln(1-q)]    (two ACT passes with scale=-1,bias=1,
                                 each with accum_out -> per-partition sums
                                 s1p, s1q; so sum(ln(1-p)) - sum(ln(1-q))
                                 costs no DVE/PE work)
    W  = L - L1                 (one DVE sub over [2F])
    d  = W[:, :F] - W[:, F:]    = (ln p - ln q) - (ln(1-p) - ln(1-q))
    m  = p16 * d                (p16 cast on GpSimd)
    sum(kl) = sum(s1p) - sum(s1q) + sum(m)

Sharding: element axis split evenly across 8 cores; host sums in fp64.
"""

import numpy as np

import concourse.bass as bass
import concourse.bacc as bacc
import concourse.mybir as mybir
from concourse import bass_utils
from concourse.tile import TileContext

N = 33554432
NCORES = 8
PER = N // NCORES   # 4194304 elements per core per tensor
P = 128
CPART = PER // P    # 32768 free-dim columns per tensor per core
NRED = 512          # one PSUM bank of fp32: matmul free-dim chunk

AF = mybir.ActivationFunctionType
OP = mybir.AluOpType
DT = mybir.dt

CHUNKS = [512, 512, 1024] + [3072] * 9 + [1024, 1024, 512, 512]
assert sum(CHUNKS) == CPART
NCH = len(CHUNKS)

_NC_CACHE = {}


def _build_nc():
    nc = bacc.Bacc("TRN2", target_bir_lowering=False, debug=False,
                   num_devices=NCORES)
    inp = nc.dram_tensor("input", [PER], DT.float32, kind="ExternalInput")
    tgt = nc.dram_tensor("target", [PER], DT.float32, kind="ExternalInput")
    out = nc.dram_tensor("partials", [NRED], DT.float32,
                         kind="ExternalOutput")
    aout = nc.dram_tensor("asums", [P * 2 * NCH], DT.float32,
                          kind="ExternalOutput")

    p_flat = inp.ap()
    q_flat = tgt.ap()
    out_view = out.ap().rearrange("(o n) -> o n", o=1)
    aout_view = aout.ap().rearrange("(p c) -> p c", p=P)

    n_mm = CPART // NRED  # 64 matmuls accumulate sum(p*d)

    with TileContext(nc) as tc:
        with (
            tc.tile_pool(name="io32", bufs=2) as io32,
            tc.tile_pool(name="f16", bufs=2) as f16,
            tc.tile_pool(name="cst", bufs=1) as cst,
            tc.tile_pool(name="ps", bufs=1, space="PSUM") as psp,
        ):
            ones = cst.tile([P, 1], DT.float16, tag="ones")
            nc.vector.memset(ones[:], 1.0)
            acc = psp.tile([1, NRED], DT.float32, tag="acc")
            osb = cst.tile([1, NRED], DT.float32, tag="osb")
            # s1[:, 0:NCH] = per-chunk accums of ln(1-p),
            # s1[:, NCH:2*NCH] = of ln(1-q)
            s1 = cst.tile([P, 2 * NCH], DT.float32, tag="s1")

            # Dummy 1-element Ln at t=0 so the ACT table load happens while
            # the first DMA is still in flight.
            warm = cst.tile([1, 1], DT.float32, tag="warm")
            nc.vector.memset(warm[:], 0.5)
            nc.scalar.activation(osb[0:1, 0:1], warm[:], AF.Ln)

            # --- timing probes in the warmup shadow (results unused but
            # kept live by writing into osb, which is fully rewritten at
            # the end). Used to read real HW costs from the trace.
            prb = cst.tile([P, 512], DT.float16, tag="prb")
            nc.vector.memset(prb[:], 1.0)
            prr = cst.tile([P, 64], DT.float16, tag="prr")
            nc.vector.tensor_reduce(
                prr[:], prb[:].rearrange("p (g k) -> p g k", k=8),
                mybir.AxisListType.X, OP.mult)
            nc.vector.tensor_copy(osb[0:1, 1:2], prr[0:1, 0:1])
            psd = psp.tile([P, 128], DT.float32, tag="psd")
            nc.tensor.matmul(psd[:, :], prb[:, 0:128], prb[:, 128:256],
                             start=True, stop=True)
            nc.vector.tensor_copy(osb[0:1, 2:3], psd[0:1, 0:1])

            mm = 0
            base = 0
            for ci, F in enumerate(CHUNKS):
                pq = io32.tile([P, 2 * F], DT.float32, tag="pq")
                nc.sync.dma_start(
                    pq[:, 0:F],
                    p_flat[base:base + P * F].rearrange("(p f) -> p f", p=P))
                nc.sync.dma_start(
                    pq[:, F:2 * F],
                    q_flat[base:base + P * F].rearrange("(p f) -> p f", p=P))
                base += P * F

                L = f16.tile([P, 2 * F], DT.float16, tag="L")
                nc.scalar.activation(L[:], pq[:], AF.Ln)
                L1 = f16.tile([P, 2 * F], DT.float16, tag="L1")
                nc.scalar.activation(L1[:, 0:F], pq[:, 0:F], AF.Ln,
                                     bias=1.0, scale=-1.0,
                                     accum_out=s1[:, ci:ci + 1])
                nc.scalar.activation(L1[:, F:2 * F], pq[:, F:2 * F], AF.Ln,
                                     bias=1.0, scale=-1.0,
                                     accum_out=s1[:, NCH + ci:NCH + ci + 1])

                # p16 cast on GpSimd (clamp below 1.0 is harmless here and
                # uses the walrus-verified TensorScalar-min pattern)
                p16 = f16.tile([P, F], DT.float16, tag="p16")
                nc.gpsimd.tensor_scalar_min(p16[:], pq[:, 0:F],
                                            1.0 - 2.0 ** -11)

                W = f16.tile([P, 2 * F], DT.float16, tag="W")
                nc.vector.tensor_tensor(W[:], L[:], L1[:], OP.subtract)

                d = f16.tile([P, F], DT.float16, tag="d")
                nc.vector.tensor_tensor(d[:], W[:, 0:F], W[:, F:2 * F],
                                        OP.subtract)
                m = f16.tile([P, F], DT.float16, tag="m")
                nc.vector.tensor_tensor(m[:], p16[:], d[:], OP.mult)

                for c in range(F // NRED):
                    nc.tensor.matmul(
                        acc[:, :], ones[:], m[:, c * NRED:(c + 1) * NRED],
                        start=(mm == 0), stop=(mm == n_mm - 1))
                    mm += 1

            nc.vector.tensor_copy(osb[:], acc[:])
            nc.sync.dma_start(out_view[:], osb[:])
            nc.sync.dma_start(aout_view[:], s1[:])

    nc.compile()
    return nc


def _get_nc():
    if "nc" not in _NC_CACHE:
        _NC_CACHE["nc"] = _build_nc()
    return _NC_CACHE["nc"]


def kernel(input, target, _trace=False):
    input = np.ascontiguousarray(np.asarray(input), dtype=np.float32)
    target = np.ascontiguousarray(np.asarray(target), dtype=np.float32)
    nc = _get_nc()
    in_maps = [
        {
            "input": input[c * PER:(c + 1) * PER],
            "target": target[c * PER:(c + 1) * PER],
        }
        for c in range(NCORES)
    ]
    res = bass_utils.run_bass_kernel_spmd(
        nc, in_maps, core_ids=list(range(NCORES)), trace=_trace)
    total = np.float64(0.0)
    for c in range(NCORES):
        total += res.results[c]["partials"].astype(np.float64).sum()
        a = res.results[c]["asums"].astype(np.float64).reshape(P, 2 * NCH)
        total += a[:, 0:NCH].sum() - a[:, NCH:2 * NCH].sum()
    out = np.asarray(total, dtype=np.float32)
    if _trace:
        return out, res
    return out


# revision 7
# speedup vs baseline: 3.0752x; 3.0752x over previous
"""Binary KL divergence sum on 8 Trainium2 NeuronCores.

Reference math (per element, summed over all 2**25 elements):
    kl = p*(ln p - ln q) + (1-p)*(ln(1-p) - ln(1-q))

Decomposition used here (ACT does 4 logs/elem-pair; DVE ops minimized):
    L  = [ln p | ln q]          (one ACT pass over [p|q], fp32 in)
    L1 = [ln(1-p) | ln(1-q)]    (two ACT passes with scale=-1,bias=1,
                                 each with accum_out -> per-partition sums
                                 s1p, s1q; so sum(ln(1-p)) - sum(ln(1-q))
                                 costs no DVE/PE work)
    W  = L - L1                 (one DVE sub over [2F])
    d  = W[:, :F] - W[:, F:]    = (ln p - ln q) - (ln(1-p) - ln(1-q))
    m  = p16 * d                (p16 cast on GpSimd)
    sum(kl) = sum(s1p) - sum(s1q) + sum(m)

Sharding: element axis split evenly across 8 cores; host sums in fp64.
"""

import numpy as np

import concourse.bass as bass
import concourse.bacc as bacc
import concourse.mybir as mybir
from concourse import bass_utils
from concourse.tile import TileContext

N = 33554432
NCORES = 8
PER = N // NCORES   # 4194304 elements per core per tensor
P = 128
CPART = PER // P    # 32768 free-dim columns per tensor per core
NRED = 512          # one PSUM bank of fp32: matmul free-dim chunk

AF = mybir.ActivationFunctionType
OP = mybir.AluOpType
DT = mybir.dt

CHUNKS = [512, 512, 1024] + [3072] * 9 + [1024, 1024, 512, 512]
assert sum(CHUNKS) == CPART
NCH = len(CHUNKS)

_NC_CACHE = {}


def _build_nc():
    nc = bacc.Bacc("TRN2", target_bir_lowering=False, debug=False,
                   num_devices=NCORES)
    inp = nc.dram_tensor("input", [PER], DT.float32, kind="ExternalInput")
    tgt = nc.dram_tensor("target", [PER], DT.float32, kind="ExternalInput")
    out = nc.dram_tensor("partials", [NRED], DT.float32,
                         kind="ExternalOutput")
    aout = nc.dram_tensor("asums", [P * 2 * NCH], DT.float32,
                          kind="ExternalOutput")

    p_flat = inp.ap()
    q_flat = tgt.ap()
    out_view = out.ap().rearrange("(o n) -> o n", o=1)
    aout_view = aout.ap().rearrange("(p c) -> p c", p=P)

    n_mm = CPART // NRED  # 64 matmuls accumulate sum(p*d)

    with TileContext(nc) as tc:
        with (
            tc.tile_pool(name="io32", bufs=2) as io32,
            tc.tile_pool(name="f16", bufs=2) as f16,
            tc.tile_pool(name="cst", bufs=1) as cst,
            tc.tile_pool(name="ps", bufs=1, space="PSUM") as psp,
        ):
            ones = cst.tile([P, 1], DT.float16, tag="ones")
            nc.vector.memset(ones[:], 1.0)
            acc = psp.tile([1, NRED], DT.float32, tag="acc")
            osb = cst.tile([1, NRED], DT.float32, tag="osb")
            # s1[:, 0:NCH] = per-chunk accums of ln(1-p),
            # s1[:, NCH:2*NCH] = of ln(1-q)
            s1 = cst.tile([P, 2 * NCH], DT.float32, tag="s1")

            # Dummy 1-element Ln at t=0 so the ACT table load happens while
            # the first DMA is still in flight.
            warm = cst.tile([1, 1], DT.float32, tag="warm")
            nc.vector.memset(warm[:], 0.5)
            nc.scalar.activation(osb[0:1, 0:1], warm[:], AF.Ln)

            # --- timing probes in the warmup shadow (results unused but
            # kept live by writing into osb, which is fully rewritten at
            # the end). Used to read real HW costs from the trace.
            prb = cst.tile([P, 512], DT.float16, tag="prb")
            nc.vector.memset(prb[:], 1.0)
            prr = cst.tile([P, 64], DT.float16, tag="prr")
            nc.vector.tensor_reduce(
                prr[:], prb[:].rearrange("p (g k) -> p g k", k=8),
                mybir.AxisListType.X, OP.mult)
            nc.vector.tensor_copy(osb[0:1, 1:2], prr[0:1, 0:1])
            psd = psp.tile([P, 128], DT.float32, tag="psd")
            nc.tensor.matmul(psd[:, :], prb[:, 0:128], prb[:, 128:256],
                             start=True, stop=True)
            nc.vector.tensor_copy(osb[0:1, 2:3], psd[0:1, 0:1])

            mm = 0
            base = 0
            for ci, F in enumerate(CHUNKS):
                pq = io32.tile([P, 2 * F], DT.float32, tag="pq")
                nc.sync.dma_start(
                    pq[:, 0:F],
                    p_flat[base:base + P * F].rearrange("(p f) -> p f", p=P))
                nc.sync.dma_start(
                    pq[:, F:2 * F],
                    q_flat[base:base + P * F].rearrange("(p f) -> p f", p=P))
                base += P * F

                L = f16.tile([P, 2 * F], DT.float16, tag="L")
                nc.scalar.activation(L[:], pq[:], AF.Ln)
                L1 = f16.tile([P, 2 * F], DT.float16, tag="L1")
                nc.scalar.activation(L1[:, 0:F], pq[:, 0:F], AF.Ln,
                                     bias=1.0, scale=-1.0,
                                     accum_out=s1[:, ci:ci + 1])
                nc.scalar.activation(L1[:, F:2 * F], pq[:, F:2 * F], AF.Ln,
                                     bias=1.0, scale=-1.0,
                                     accum_out=s1[:, NCH + ci:NCH + ci + 1])

                # p16 cast on DVE (2x_2p mode works with fp32 input; GpSimd
                # measured ~17x below its modeled rate and also poisons DVE
                # throughput via SBUF contention, so it gets no bulk work)
                p16 = f16.tile([P, F], DT.float16, tag="p16")
                nc.vector.tensor_copy(p16[:], pq[:, 0:F])

                W = f16.tile([P, 2 * F], DT.float16, tag="W")
                nc.vector.tensor_tensor(W[:], L[:], L1[:], OP.subtract)

                d = f16.tile([P, F], DT.float16, tag="d")
                nc.vector.tensor_tensor(d[:], W[:, 0:F], W[:, F:2 * F],
                                        OP.subtract)
                m = f16.tile([P, F], DT.float16, tag="m")
                nc.vector.tensor_tensor(m[:], p16[:], d[:], OP.mult)

                for c in range(F // NRED):
                    nc.tensor.matmul(
                        acc[:, :], ones[:], m[:, c * NRED:(c + 1) * NRED],
                        start=(mm == 0), stop=(mm == n_mm - 1))
                    mm += 1

            nc.vector.tensor_copy(osb[:], acc[:])
            nc.sync.dma_start(out_view[:], osb[:])
            nc.sync.dma_start(aout_view[:], s1[:])

    nc.compile()
    return nc


def _get_nc():
    if "nc" not in _NC_CACHE:
        _NC_CACHE["nc"] = _build_nc()
    return _NC_CACHE["nc"]


def kernel(input, target, _trace=False):
    input = np.ascontiguousarray(np.asarray(input), dtype=np.float32)
    target = np.ascontiguousarray(np.asarray(target), dtype=np.float32)
    nc = _get_nc()
    in_maps = [
        {
            "input": input[c * PER:(c + 1) * PER],
            "target": target[c * PER:(c + 1) * PER],
        }
        for c in range(NCORES)
    ]
    res = bass_utils.run_bass_kernel_spmd(
        nc, in_maps, core_ids=list(range(NCORES)), trace=_trace)
    total = np.float64(0.0)
    for c in range(NCORES):
        total += res.results[c]["partials"].astype(np.float64).sum()
        a = res.results[c]["asums"].astype(np.float64).reshape(P, 2 * NCH)
        total += a[:, 0:NCH].sum() - a[:, NCH:2 * NCH].sum()
    out = np.asarray(total, dtype=np.float32)
    if _trace:
        return out, res
    return out


# revision 8
# speedup vs baseline: 3.3795x; 1.0989x over previous
"""Binary KL divergence sum on 8 Trainium2 NeuronCores.

Reference math (per element, summed over all 2**25 elements):
    kl = p*(ln p - ln q) + (1-p)*(ln(1-p) - ln(1-q))

Rewritten with t1 = ln p - ln q, t2 = ln(1-p) - ln(1-q):
    kl = t2 + p*(t1 - t2)
    sum(kl) = sum(t2) + sum(p * (t1 - t2))

The Scalar (ACT) engine is the critical path: 4 Ln evaluations per
element pair at 1 elem/cycle/partition @1.2GHz = ~109us/core minimum.
Everything else is arranged to stay off the Scalar queue:
  - two activations per chunk (Ln(pq), Ln(1-pq)), no accum_out
    (accum_out measured ~1.2us/use of extra Scalar-queue time)
  - both reduction streams (t2 and m) summed by PE ones-matmuls
  - DVE ops sized to ~3F cycles/chunk, all in 2x/2x_2p perf modes
  - GpSimd gets no bulk work (measured ~17x below modeled rate and
    poisons DVE throughput via SBUF contention)

Per-core pipeline (chunks of [128, 2F], p left, q right):
  DMA : p -> pq[:, :F], q -> pq[:, F:]            (fp32)
  ACT : L  = Ln(pq)           -> fp16             (2F cycles)
  ACT : L1 = Ln(1 - pq)       -> fp16             (2F cycles)
  DVE : p16 = cast(pq[:, :F]) -> fp16             (2x_2p)
  DVE : W  = L - L1                               (fp16 2x)
  DVE : t2 = L1[:, :F] - L1[:, F:]  (in-place -> L1[:, :F])
  DVE : d  = W[:, :F] - W[:, F:]    (in-place -> W[:, :F])
  DVE : m  = p16 * d                (in-place -> W[:, F:])
  PE  : acc[1, 512] += ones.T @ t2 ; += ones.T @ m
Host sums the 512 fp32 partials per core in fp64.
"""

import numpy as np

import concourse.bass as bass
import concourse.bacc as bacc
import concourse.mybir as mybir
from concourse import bass_utils
from concourse.tile import TileContext

N = 33554432
NCORES = 8
PER = N // NCORES   # 4194304 elements per core per tensor
P = 128
CPART = PER // P    # 32768 free-dim columns per tensor per core
NRED = 512          # one PSUM bank of fp32: matmul free-dim chunk

AF = mybir.ActivationFunctionType
OP = mybir.AluOpType
DT = mybir.dt

CHUNKS = [1024, 1024] + [4096] * 7 + [1024, 1024]
assert sum(CHUNKS) == CPART
NCH = len(CHUNKS)

_NC_CACHE = {}


def _build_nc():
    nc = bacc.Bacc("TRN2", target_bir_lowering=False, debug=False,
                   num_devices=NCORES)
    inp = nc.dram_tensor("input", [PER], DT.float32, kind="ExternalInput")
    tgt = nc.dram_tensor("target", [PER], DT.float32, kind="ExternalInput")
    out = nc.dram_tensor("partials", [NRED], DT.float32,
                         kind="ExternalOutput")

    p_flat = inp.ap()
    q_flat = tgt.ap()
    out_view = out.ap().rearrange("(o n) -> o n", o=1)

    n_mm = 2 * (CPART // NRED)  # 128: t2-sums and m-sums share one acc

    with TileContext(nc) as tc:
        with (
            tc.tile_pool(name="io32", bufs=2) as io32,
            tc.tile_pool(name="f16", bufs=2) as f16,
            tc.tile_pool(name="cst", bufs=1) as cst,
            tc.tile_pool(name="ps", bufs=1, space="PSUM") as psp,
        ):
            ones = cst.tile([P, 1], DT.float16, tag="ones")
            nc.vector.memset(ones[:], 1.0)
            acc = psp.tile([1, NRED], DT.float32, tag="acc")
            osb = cst.tile([1, NRED], DT.float32, tag="osb")

            # Dummy 1-element Ln at t=0 so the ACT table load happens while
            # the first DMA is still in flight.
            warm = cst.tile([1, 1], DT.float32, tag="warm")
            nc.vector.memset(warm[:], 0.5)
            nc.scalar.activation(osb[0:1, 0:1], warm[:], AF.Ln)

            mm = 0
            base = 0
            for F in CHUNKS:
                pq = io32.tile([P, 2 * F], DT.float32, tag="pq")
                nc.sync.dma_start(
                    pq[:, 0:F],
                    p_flat[base:base + P * F].rearrange("(p f) -> p f", p=P))
                nc.sync.dma_start(
                    pq[:, F:2 * F],
                    q_flat[base:base + P * F].rearrange("(p f) -> p f", p=P))
                base += P * F

                L = f16.tile([P, 2 * F], DT.float16, tag="L")
                nc.scalar.activation(L[:], pq[:], AF.Ln)
                L1 = f16.tile([P, 2 * F], DT.float16, tag="L1")
                nc.scalar.activation(L1[:], pq[:], AF.Ln,
                                     bias=1.0, scale=-1.0)

                p16 = f16.tile([P, F], DT.float16, tag="p16")
                nc.vector.tensor_copy(p16[:], pq[:, 0:F])

                W = f16.tile([P, 2 * F], DT.float16, tag="W")
                nc.vector.tensor_tensor(W[:], L[:], L1[:], OP.subtract)

                # t2 in-place into L1's p-half (W already consumed L1)
                nc.vector.tensor_tensor(L1[:, 0:F], L1[:, 0:F],
                                        L1[:, F:2 * F], OP.subtract)
                # d in-place into W's p-half
                nc.vector.tensor_tensor(W[:, 0:F], W[:, 0:F],
                                        W[:, F:2 * F], OP.subtract)
                # m = p16 * d in-place into W's q-half
                nc.vector.tensor_tensor(W[:, F:2 * F], p16[:], W[:, 0:F],
                                        OP.mult)

                for c in range(F // NRED):
                    nc.tensor.matmul(
                        acc[:, :], ones[:],
                        L1[:, c * NRED:(c + 1) * NRED],
                        start=(mm == 0), stop=(mm == n_mm - 1))
                    mm += 1
                for c in range(F // NRED):
                    nc.tensor.matmul(
                        acc[:, :], ones[:],
                        W[:, F + c * NRED:F + (c + 1) * NRED],
                        start=(mm == 0), stop=(mm == n_mm - 1))
                    mm += 1

            nc.vector.tensor_copy(osb[:], acc[:])
            nc.sync.dma_start(out_view[:], osb[:])

    nc.compile()
    return nc


def _get_nc():
    if "nc" not in _NC_CACHE:
        _NC_CACHE["nc"] = _build_nc()
    return _NC_CACHE["nc"]


def kernel(input, target, _trace=False):
    input = np.ascontiguousarray(np.asarray(input), dtype=np.float32)
    target = np.ascontiguousarray(np.asarray(target), dtype=np.float32)
    nc = _get_nc()
    in_maps = [
        {
            "input": input[c * PER:(c + 1) * PER],
            "target": target[c * PER:(c + 1) * PER],
        }
        for c in range(NCORES)
    ]
    res = bass_utils.run_bass_kernel_spmd(
        nc, in_maps, core_ids=list(range(NCORES)), trace=_trace)
    total = np.float64(0.0)
    for c in range(NCORES):
        total += res.results[c]["partials"].astype(np.float64).sum()
    out = np.asarray(total, dtype=np.float32)
    if _trace:
        return out, res
    return out


# revision 9
# speedup vs baseline: 3.7762x; 1.1174x over previous
"""Binary KL divergence sum on 8 Trainium2 NeuronCores.

Reference math (per element, summed over all 2**25 elements):
    kl = p*(ln p - ln q) + (1-p)*(ln(1-p) - ln(1-q))

Rewritten with t1 = ln p - ln q, t2 = ln(1-p) - ln(1-q):
    kl = t2 + p*(t1 - t2)
    sum(kl) = sum(t2) + sum(p * (t1 - t2))

The Scalar (ACT) engine is the critical path: 4 Ln evaluations per
element pair at 1 elem/cycle/partition @1.2GHz = ~109us/core minimum.
Everything else is arranged to stay off the Scalar queue:
  - two activations per chunk (Ln(pq), Ln(1-pq)), no accum_out
    (accum_out measured ~1.2us/use of extra Scalar-queue time)
  - both reduction streams (t2 and m) summed by PE ones-matmuls
  - DVE ops sized to ~3F cycles/chunk, all in 2x/2x_2p perf modes
  - GpSimd gets no bulk work (measured ~17x below modeled rate and
    poisons DVE throughput via SBUF contention)

Per-core pipeline (chunks of [128, 2F], p left, q right):
  DMA : p -> pq[:, :F], q -> pq[:, F:]            (fp32)
  ACT : L  = Ln(pq)           -> fp16             (2F cycles)
  ACT : L1 = Ln(1 - pq)       -> fp16             (2F cycles)
  DVE : p16 = cast(pq[:, :F]) -> fp16             (2x_2p)
  DVE : W  = L - L1                               (fp16 2x)
  DVE : t2 = L1[:, :F] - L1[:, F:]  (in-place -> L1[:, :F])
  DVE : d  = W[:, :F] - W[:, F:]    (in-place -> W[:, :F])
  DVE : m  = p16 * d                (in-place -> W[:, F:])
  PE  : acc[1, 512] += ones.T @ t2 ; += ones.T @ m
Host sums the 512 fp32 partials per core in fp64.
"""

import numpy as np

import concourse.bass as bass
import concourse.bacc as bacc
import concourse.mybir as mybir
from concourse import bass_utils
from concourse.tile import TileContext

N = 33554432
NCORES = 8
PER = N // NCORES   # 4194304 elements per core per tensor
P = 128
CPART = PER // P    # 32768 free-dim columns per tensor per core
NRED = 512          # one PSUM bank of fp32: matmul free-dim chunk

AF = mybir.ActivationFunctionType
OP = mybir.AluOpType
DT = mybir.dt

CHUNKS = [512, 512, 1024, 2048] + [4096] * 6 + [2048, 1024, 512, 512]
assert sum(CHUNKS) == CPART
NCH = len(CHUNKS)

_NC_CACHE = {}


def _build_nc():
    nc = bacc.Bacc("TRN2", target_bir_lowering=False, debug=False,
                   num_devices=NCORES)
    inp = nc.dram_tensor("input", [PER], DT.float32, kind="ExternalInput")
    tgt = nc.dram_tensor("target", [PER], DT.float32, kind="ExternalInput")
    out = nc.dram_tensor("partials", [NRED], DT.float32,
                         kind="ExternalOutput")

    p_flat = inp.ap()
    q_flat = tgt.ap()
    out_view = out.ap().rearrange("(o n) -> o n", o=1)

    n_mm = 2 * (CPART // NRED)  # 128: t2-sums and m-sums share one acc

    with TileContext(nc) as tc:
        with (
            tc.tile_pool(name="io32", bufs=3) as io32,
            tc.tile_pool(name="f16", bufs=2) as f16,
            tc.tile_pool(name="cst", bufs=1) as cst,
            tc.tile_pool(name="ps", bufs=1, space="PSUM") as psp,
        ):
            ones = cst.tile([P, 1], DT.float16, tag="ones")
            nc.vector.memset(ones[:], 1.0)
            acc = psp.tile([1, NRED], DT.float32, tag="acc")
            osb = cst.tile([1, NRED], DT.float32, tag="osb")

            # Dummy 1-element Ln at t=0 so the ACT table load happens while
            # the first DMA is still in flight.
            warm = cst.tile([1, 1], DT.float32, tag="warm")
            nc.vector.memset(warm[:], 0.5)
            nc.scalar.activation(osb[0:1, 0:1], warm[:], AF.Ln)

            mm = 0
            base = 0
            for F in CHUNKS:
                pq = io32.tile([P, 2 * F], DT.float32, tag="pq")
                nc.sync.dma_start(
                    pq[:, 0:F],
                    p_flat[base:base + P * F].rearrange("(p f) -> p f", p=P))
                nc.sync.dma_start(
                    pq[:, F:2 * F],
                    q_flat[base:base + P * F].rearrange("(p f) -> p f", p=P))
                base += P * F

                L = f16.tile([P, 2 * F], DT.float16, tag="L")
                nc.scalar.activation(L[:], pq[:], AF.Ln)
                L1 = f16.tile([P, 2 * F], DT.float16, tag="L1")
                nc.scalar.activation(L1[:], pq[:], AF.Ln,
                                     bias=1.0, scale=-1.0)

                p16 = f16.tile([P, F], DT.float16, tag="p16")
                nc.vector.tensor_copy(p16[:], pq[:, 0:F])

                # t1 = ln p - ln q, in-place into L's p-half (L has a
                # single DVE consumer, so ACT signals once per activation)
                nc.vector.tensor_tensor(L[:, 0:F], L[:, 0:F],
                                        L[:, F:2 * F], OP.subtract)
                # t2 = ln(1-p) - ln(1-q), in-place into L1's p-half
                nc.vector.tensor_tensor(L1[:, 0:F], L1[:, 0:F],
                                        L1[:, F:2 * F], OP.subtract)
                # d = t1 - t2, in-place into t1
                nc.vector.tensor_tensor(L[:, 0:F], L[:, 0:F],
                                        L1[:, 0:F], OP.subtract)
                # m = p16 * d, into L's q-half
                nc.vector.tensor_tensor(L[:, F:2 * F], p16[:], L[:, 0:F],
                                        OP.mult)

                for c in range(F // NRED):
                    nc.tensor.matmul(
                        acc[:, :], ones[:],
                        L1[:, c * NRED:(c + 1) * NRED],
                        start=(mm == 0), stop=(mm == n_mm - 1))
                    mm += 1
                for c in range(F // NRED):
                    nc.tensor.matmul(
                        acc[:, :], ones[:],
                        L[:, F + c * NRED:F + (c + 1) * NRED],
                        start=(mm == 0), stop=(mm == n_mm - 1))
                    mm += 1

            nc.vector.tensor_copy(osb[:], acc[:])
            nc.sync.dma_start(out_view[:], osb[:])

    nc.compile()
    return nc


def _get_nc():
    if "nc" not in _NC_CACHE:
        _NC_CACHE["nc"] = _build_nc()
    return _NC_CACHE["nc"]


def kernel(input, target, _trace=False):
    input = np.ascontiguousarray(np.asarray(input), dtype=np.float32)
    target = np.ascontiguousarray(np.asarray(target), dtype=np.float32)
    nc = _get_nc()
    in_maps = [
        {
            "input": input[c * PER:(c + 1) * PER],
            "target": target[c * PER:(c + 1) * PER],
        }
        for c in range(NCORES)
    ]
    res = bass_utils.run_bass_kernel_spmd(
        nc, in_maps, core_ids=list(range(NCORES)), trace=_trace)
    total = np.float64(0.0)
    for c in range(NCORES):
        total += res.results[c]["partials"].astype(np.float64).sum()
    out = np.asarray(total, dtype=np.float32)
    if _trace:
        return out, res
    return out
